# revision 40
# baseline (speedup 1.0000x reference)
"""CrossAttentionFusion kernel for Trainium2 (8 NeuronCores, Bass/Tile).

Computation (matches the reference nn.Module):
  image_proj = relu(BN(1x1conv(image_features, image_w)))   # (B,128,H,W)
  lidar_proj = relu(BN(1x1conv(lidar_features, lidar_w)))   # (B,128,H,W)
  per (batch, 2048-pixel chunk): q = image_proj, k = v = lidar_proj
  attn_out = softmax(q k^T / sqrt(128)) @ k
  out = w0 * image_proj + w1 * attn_out,  w = softmax(modality_weights)

Sharding: the 16 independent (batch, chunk) attention problems are
distributed 2-per-core across 8 cores; each core computes the projections
for its own pixels.  Host gathers the 8 outputs.

v2 design (all-bf16 data path):
  - Inputs and conv weights are cast to bf16 on the host (BN scale folded
    into the weights, w0 folded into the image weights/bias); f32 PSUM
    accumulation keeps the projections accurate; ACT applies bias+relu and
    writes qTb/kTb directly as bf16 (no separate downcast pass).
  - Scores: sT[k,q] = kTb_slice.T @ qTb (bf16 in, f32 psum), exp split
    between ACT (true exp -> bf16, 13 slices) and DVE (Schraudolph int16
    bit-trick, 3 slices) -- ~3% weight error that cancels through the
    shared softmax denominator.
  - exp writes two k-slices into one [128,2048] pair tile, so the softmax
    denominator accumulates with 7 double-width DVE adds per q-block; one
    (1/w1)-matrix matmul partition-reduces + broadcasts it, reciprocal on
    DVE gives linv = w1/L, and the output is po*linv + qTb (2 DVE ops).
  - AV: kp (pixel-major bf16 via PE transposes) @ et, f32 psum accum.
  - Scheduling: one flat software pipeline.  Scores use a 3-deep PSUM
    ring (AV accum uses the other 2 banks); the second projection halves,
    transposes, the previous q-block's denominator + drain, and the next
    unit's whole projection are injected into the running q-block's slice
    loop so no engine sees a block boundary; the lookahead-tail AV
    matmuls are deferred into the drain so they never block the next
    block's first scores in the PE queue.
  - DMAs: 2 packed const DMAs + 2 per (unit, modality) issued up front in
    consumption order (the serial ~650ns/descriptor sync-queue issue rate
    was the old 22us startup bottleneck); dummy warm-up matmuls keep the
    PE HAM warm through the DMA head; per-q-block output DMAs and a
    half-granular fast drain on the last q-block shrink the tail.
"""

import math
import os
import sys
from contextlib import ExitStack

import numpy as np

sys.path.insert(0, "/opt/trn_rl_repo")

import concourse.bass as bass  # noqa: E402
import concourse.tile as tile  # noqa: E402
from concourse import bacc, mybir  # noqa: E402
from concourse.bass import ds, ts  # noqa: E402
from concourse.bass_utils import run_bass_kernel_spmd  # noqa: E402

F32 = mybir.dt.float32
BF16 = mybir.dt.bfloat16
I16 = mybir.dt.int16

B, CL, CI, CO = 2, 256, 512, 128
H = W = 128
P = H * W                    # 16384 pixels per batch
CHUNK = 2048                 # attention chunk (pixels)
NCH = P // CHUNK             # 8 chunks per batch
NCORES = 8
UPC = (B * NCH) // NCORES    # units (b,chunk) per core = 2
EPS = 1e-5
QB = 1024                    # q-block width (2 matmul halves of 512)
NQB = CHUNK // QB            # 2
KSL = CHUNK // 128           # 16 k-pixel slices per chunk
NCI_I = CI // 128            # 4 contraction slices for image proj
NCI_L = CL // 128            # 2 for lidar proj

# exp engine per k-slice index: A=ACT true exp, D=DVE Schraudolph.
# one pattern per (unit,qb), comma-separated; a single pattern applies
# to all q-blocks
_EXP = os.environ.get("K_EXP_ENG", "AADAAAADAAAADAAA").split(",")
EXP_PAT = (_EXP * 4)[:4] if len(_EXP) in (1, 2, 4) else _EXP
ADD_LAG = int(os.environ.get("K_ADD_LAG", "3"))
LOOKAHEAD = int(os.environ.get("K_LOOKAHEAD", "3"))
K_WARM = int(os.environ.get("K_WARM", "8"))
# proj relu engine, 8 chars: u0[kh0,kh1,qh0,qh1], u1[...]  A=ACT, D=DVE
RELU_ENG = os.environ.get("K_RELU_ENG", "AAAAAAAA")
ET_BUFS = int(os.environ.get("K_ET_BUFS", "10"))
MM_BUFS = int(os.environ.get("K_MM_BUFS", "3"))
AV_BUFS = int(os.environ.get("K_AV_BUFS", "1"))
AUX_BUFS = int(os.environ.get("K_AUX_BUFS", "0"))  # 0 = share mm ring
DRAIN_AT = int(os.environ.get("K_DRAIN_AT", "2"))

# Schraudolph constants for bf16-bits-in-int16 exp approximation
# (DVE float->int conversion truncates; C tuned for that):
#   exp(x) ~= bitcast_bf16(int16(x * 128/ln2 + (127*128 - C)))
SCH_A = 128.0 / math.log(2.0)
SCH_C = 5.0

# cf (f32 const) column indices
CF_IMG_B, CF_LID_B, CF_ESC, CF_SCHA, CF_SCHB = 0, 1, 2, 3, 4
# cb (bf16 const) column offsets
CB_WLID, CB_IDENT, CB_WIMG, CB_INVW1 = 0, 256, 384, 896

_PROGRAM = None              # compiled Bass program, built once per process
LAST_RESULTS = None          # BassKernelResults of the last kernel() call


def _build_program():
    nc = bacc.Bacc("TRN2", target_bir_lowering=False, debug=False,
                   num_devices=NCORES)

    cb = nc.dram_tensor("cb", [128, 1024], BF16, kind="ExternalInput").ap()
    cf = nc.dram_tensor("cf", [128, 8], F32, kind="ExternalInput").ap()
    # per-(unit,pixel-half) inputs, ci-major within the SBUF row
    xl = nc.dram_tensor("xl", [UPC, 2, 128, NCI_L, 1024], BF16,
                        kind="ExternalInput").ap()
    xi = nc.dram_tensor("xi", [UPC, 2, 128, NCI_I, 1024], BF16,
                        kind="ExternalInput").ap()
    y = nc.dram_tensor("y", [UPC, CO, CHUNK], F32, kind="ExternalOutput").ap()

    with tile.TileContext(nc) as tc, ExitStack() as ctx:
        const = ctx.enter_context(tc.tile_pool(name="const", bufs=1))
        xl_pool = ctx.enter_context(tc.tile_pool(name="xl", bufs=2))
        xi_pool = ctx.enter_context(tc.tile_pool(name="xi", bufs=2))
        kt_pool = ctx.enter_context(tc.tile_pool(name="kt", bufs=2))
        qt_pool = ctx.enter_context(tc.tile_pool(name="qt", bufs=2))
        kp_pool = ctx.enter_context(tc.tile_pool(name="kp", bufs=2))
        et_pool = ctx.enter_context(tc.tile_pool(name="et", bufs=ET_BUFS))
        s_pool = ctx.enter_context(tc.tile_pool(name="s", bufs=4))
        misc_pool = ctx.enter_context(tc.tile_pool(name="misc", bufs=4))
        res_pool = ctx.enter_context(tc.tile_pool(name="res", bufs=4))
        # PSUM: shared ring (scores/proj/transpose/denominator) + AV accum
        mm_psum = ctx.enter_context(tc.tile_pool(name="mmps", bufs=MM_BUFS, space="PSUM"))
        av_psum = ctx.enter_context(tc.tile_pool(name="avps", bufs=AV_BUFS, space="PSUM"))
        if AUX_BUFS:
            aux_psum = ctx.enter_context(tc.tile_pool(name="auxps", bufs=AUX_BUFS, space="PSUM"))
            aux_tag = "aux"
        else:
            aux_psum = mm_psum
            aux_tag = "mm"

        # ---- constants + all input DMAs, issued up front in use order ----
        cb_t = const.tile([128, 1024], BF16)
        cf_t = const.tile([128, 8], F32)
        warm = const.tile([128, 640], BF16)

        xl_ts, xi_ts = [], []
        for u in range(UPC):
            xl_ts.append(xl_pool.tile([128, NCI_L, 2048], BF16,
                                      name=f"xl_{u}", tag="xl"))
            xi_ts.append(xi_pool.tile([128, NCI_I, 2048], BF16,
                                      name=f"xi_{u}", tag="xi"))

        # Descriptor issue costs 0.7-2.9us per DMA on an engine queue, so
        # split the input DMAs across the Sync and (otherwise idle) GPSIMD
        # queues: lidar + consts on sync, image on gpsimd.
        nc.gpsimd.memset(warm[:], 0)
        nc.sync.dma_start(cb_t[:], cb)
        nc.sync.dma_start(xl_ts[0][:, :, ds(0, 1024)], xl[0, 0])
        nc.sync.dma_start(cf_t[:], cf)
        nc.sync.dma_start(xl_ts[0][:, :, ds(1024, 1024)], xl[0, 1])
        for h in range(2):
            nc.gpsimd.dma_start(xi_ts[0][:, :, ds(h * 1024, 1024)], xi[0, h])
        for u in range(1, UPC):
            for h in range(2):
                nc.sync.dma_start(xl_ts[u][:, :, ds(h * 1024, 1024)], xl[u, h])
            for h in range(2):
                nc.gpsimd.dma_start(xi_ts[u][:, :, ds(h * 1024, 1024)],
                                    xi[u, h])

        # ---- PE warm-up: keep HAM busy while input DMAs land; a dummy
        # exp pulls the lazy ACT table load into the DMA head ----
        if K_WARM:
            warm_ps = mm_psum.tile([128, 512], F32, name="warm_ps", tag="mm")
            for _ in range(K_WARM):
                nc.tensor.matmul(warm_ps[:], warm[:, ds(512, 128)],
                                 warm[:, ds(0, 512)], start=True, stop=True)
        nc.scalar.activation(warm[:, ds(1, 1)], warm[:, ds(0, 1)],
                             mybir.ActivationFunctionType.Exp)

        ident = cb_t[:, ds(CB_IDENT, 128)]
        invw1 = cb_t[:, ds(CB_INVW1, 128)]
        esc_ap = cf_t[:, ds(CF_ESC, 1)]
        scha_ap = cf_t[:, ds(CF_SCHA, 1)]
        schb_ap = cf_t[:, ds(CF_SCHB, 1)]

        kTb = [kt_pool.tile([128, CHUNK], BF16, name=f"kT_{u}", tag="kt")
               for u in range(UPC)]
        qTb = [qt_pool.tile([128, CHUNK], BF16, name=f"qT_{u}", tag="qt")
               for u in range(UPC)]
        kp = [kp_pool.tile([128, CHUNK], BF16, name=f"kp_{u}", tag="kp")
              for u in range(UPC)]

        def relu_store(dst, ps, bias_ap, eng):
            if eng == "A":
                nc.scalar.activation(dst, ps, mybir.ActivationFunctionType.Relu,
                                     bias=bias_ap)
            else:
                nc.vector.tensor_scalar(dst, ps, bias_ap, 0.0,
                                        op0=mybir.AluOpType.add,
                                        op1=mybir.AluOpType.max)

        def proj_k_half(u, half):
            """kTb[u] half = relu(wlid.T @ xlid + b), bf16."""
            psk = aux_psum.tile([128, QB], F32, name=f"psk_{u}_{half}",
                                tag=aux_tag)
            for b2 in range(2):
                blk = half * 2 + b2
                for ci in range(NCI_L):
                    nc.tensor.matmul(
                        psk[:, ts(b2, 512)],
                        cb_t[:, ds(CB_WLID + ci * 128, 128)],
                        xl_ts[u][:, ci, ds(blk * 512, 512)],
                        start=(ci == 0), stop=(ci == NCI_L - 1))
            relu_store(kTb[u][:, ts(half, QB)], psk[:],
                       cf_t[:, ds(CF_LID_B, 1)], RELU_ENG[u * 4 + half])

        def transpose_group(u, g):
            pt = aux_psum.tile([128, 1024], BF16, name=f"pt_{u}_{g}", tag=aux_tag)
            for k8 in range(8):
                nc.tensor.transpose(pt[:, ts(k8, 128)],
                                    kTb[u][:, ds(g * 1024 + k8 * 128, 128)],
                                    ident)
            nc.vector.tensor_copy(kp[u][:, ts(g, 1024)], pt[:])

        def proj_q_half(u, half):
            psq = aux_psum.tile([128, QB], F32, name=f"psq_{u}_{half}",
                                tag=aux_tag)
            for b2 in range(2):
                blk = half * 2 + b2
                for ci in range(NCI_I):
                    nc.tensor.matmul(
                        psq[:, ts(b2, 512)],
                        cb_t[:, ds(CB_WIMG + ci * 128, 128)],
                        xi_ts[u][:, ci, ds(blk * 512, 512)],
                        start=(ci == 0), stop=(ci == NCI_I - 1))
            relu_store(qTb[u][:, ts(half, QB)], psq[:],
                       cf_t[:, ds(CF_IMG_B, 1)], RELU_ENG[u * 4 + 2 + half])

        def proj_k_half0_split(u):
            # 512-wide psum+relu pieces so the first relu overlaps the
            # remaining matmuls (head latency only matters for unit 0)
            for b2 in range(2):
                ps = mm_psum.tile([128, 512], F32,
                                  name=f"psks_{u}_{b2}", tag="mm")
                for ci in range(NCI_L):
                    nc.tensor.matmul(ps[:],
                                     cb_t[:, ds(CB_WLID + ci * 128, 128)],
                                     xl_ts[u][:, ci, ds(b2 * 512, 512)],
                                     start=(ci == 0), stop=(ci == NCI_L - 1))
                relu_store(kTb[u][:, ds(b2 * 512, 512)], ps[:],
                           cf_t[:, ds(CF_LID_B, 1)], RELU_ENG[u * 4])

        def proj_q_half0_split(u):
            for b2 in range(2):
                ps = mm_psum.tile([128, 512], F32,
                                  name=f"psqs_{u}_{b2}", tag="mm")
                for ci in range(NCI_I):
                    nc.tensor.matmul(ps[:],
                                     cb_t[:, ds(CB_WIMG + ci * 128, 128)],
                                     xi_ts[u][:, ci, ds(b2 * 512, 512)],
                                     start=(ci == 0), stop=(ci == NCI_I - 1))
                relu_store(qTb[u][:, ds(b2 * 512, 512)], ps[:],
                           cf_t[:, ds(CF_IMG_B, 1)], RELU_ENG[u * 4 + 2])

        def attn(u, qb, inject=None, drain_fast=False):
            """Emit one q-block's attention; returns a drain closure that
            the caller emits later (inside the next q-block's slice loop)
            so the denominator matmuls never block the next block's
            scores in the PE queue.  `inject` maps slice index -> list of
            closures (deferred proj pieces / previous block's drain)."""
            lag = 1 if drain_fast else ADD_LAG
            po = av_psum.tile([128, QB], F32, name=f"po_{u}_{qb}", tag="av")
            # exp results: two k-slices share one [128, 2*QB] pair tile so
            # the S accumulation runs as 7 double-width DVE adds; the
            # denominator matmul reads both halves of the final acc
            pairs = [None] * (KSL // 2)
            ets = [None] * KSL

            # Ping-pong acc tiles so adds never read+write one tile.
            chain = {"acc": None, "k": 0, "tiles": [
                s_pool.tile([128, 2 * QB], BF16, name=f"Sm{t}_{u}_{qb}",
                            tag="S") for t in range(2)]}

            def s_add(p):
                if chain["acc"] is None:
                    chain["acc"] = pairs[p][:]
                    return
                dst = chain["tiles"][chain["k"] % 2]
                chain["k"] += 1
                nc.vector.tensor_add(dst[:], chain["acc"], pairs[p][:])
                chain["acc"] = dst[:]

            def av_mm(j):
                for h in range(2):
                    nc.tensor.matmul(po[:, ts(h, 512)],
                                     kp[u][:, ds(j * 128, 128)],
                                     ets[j][:, ts(h, 512)],
                                     start=(j == 0), stop=(j == KSL - 1))

            for i in range(KSL):
                if inject and i in inject:
                    for fn in inject[i]:
                        fn()
                ps = mm_psum.tile([128, QB], F32,
                                  name=f"pss_{u}_{qb}_{i}", tag="mm")
                for h in range(2):
                    nc.tensor.matmul(ps[:, ts(h, 512)],
                                     kTb[u][:, ds(i * 128, 128)],
                                     qTb[u][:, ds(qb * QB + h * 512, 512)],
                                     start=True, stop=True)
                if i % 2 == 0:
                    pairs[i // 2] = et_pool.tile(
                        [128, 2 * QB], BF16,
                        name=f"et_{u}_{qb}_{i // 2}", tag="et")
                et = pairs[i // 2][:, ds((i % 2) * QB, QB)]
                if EXP_PAT[u * NQB + qb][i] == "A":
                    nc.scalar.activation(et, ps[:],
                                         mybir.ActivationFunctionType.Exp,
                                         scale=esc_ap)
                else:
                    nc.vector.tensor_scalar(et.bitcast(I16), ps[:],
                                            scha_ap, schb_ap,
                                            op0=mybir.AluOpType.mult,
                                            op1=mybir.AluOpType.add)
                ets[i] = et
                a = i - lag
                if 0 <= a and a % 2 == 1:
                    s_add(a // 2)
                j = i - LOOKAHEAD
                if 0 <= j:
                    av_mm(j)

            def drain():
                # lookahead-tail AVs + tail adds, deferred so the next
                # q-block's first scores aren't queued behind them
                for i in range(KSL, KSL + max(LOOKAHEAD, lag + 1)):
                    a = i - lag
                    if a < KSL and a % 2 == 1:
                        s_add(a // 2)
                    j = i - LOOKAHEAD
                    if j < KSL:
                        av_mm(j)
                S = chain["acc"]
                # denominator: PE broadcast-sum (1/w1)^T over both halves
                # of the pair-acc, then reciprocal
                pl = aux_psum.tile([128, QB], F32, name=f"pl_{u}_{qb}",
                                   tag=aux_tag)
                for half in range(2):
                    for h in range(2):
                        nc.tensor.matmul(pl[:, ts(h, 512)], invw1,
                                         S[:, ds(half * QB + h * 512, 512)],
                                         start=(half == 0), stop=(half == 1))
                nhalf = 2 if drain_fast else 1
                wd = QB // nhalf
                for h in range(nhalf):
                    linv = misc_pool.tile([128, wd], F32,
                                          name=f"linv_{u}_{qb}_{h}",
                                          tag="linv")
                    nc.vector.reciprocal_approx_fast(linv[:], pl[:, ts(h, wd)])
                    tmp = misc_pool.tile([128, wd], F32,
                                         name=f"tmp_{u}_{qb}_{h}", tag="tmp")
                    nc.vector.tensor_mul(tmp[:], po[:, ts(h, wd)], linv[:])
                    res = res_pool.tile([128, wd], F32,
                                        name=f"res_{u}_{qb}_{h}", tag="res")
                    nc.vector.tensor_add(res[:], tmp[:],
                                         qTb[u][:, ds(qb * QB + h * wd, wd)])
                    nc.sync.dma_start(y[u][:, ds(qb * QB + h * wd, wd)],
                                      res[:])
            return drain

        def warm_mms(n):
            # dependency-free matmuls bridging PE idle gaps in the head so
            # the HAM clock-gate stays at 2.4GHz
            for _ in range(n):
                nc.tensor.matmul(warm_ps[:], warm[:, ds(512, 128)],
                                 warm[:, ds(0, 512)], start=True, stop=True)

        # software pipeline: first half-projections up front, the rest of
        # the projection work and each q-block's drain injected into the
        # following q-block's slice loop so PE/ACT/DVE never see a block
        # boundary bubble
        proj_k_half(0, 0)
        warm_mms(3)
        proj_q_half(0, 0)
        warm_mms(3)
        # relu-consumed pieces (psk/psq) are injected right after the DVE
        # exp slices {2,7,12} so their ACT relu fills the hole the D slice
        # leaves in the exp stream instead of delaying it; transpose
        # groups are DVE-consumed and slot anywhere
        d00 = attn(0, 0, inject={
            2: [lambda: proj_k_half(0, 1)],
            3: [lambda: transpose_group(0, 0)],
            7: [lambda: proj_q_half(0, 1)],
            9: [lambda: transpose_group(0, 1)],
        })
        def merge(*dicts):
            out = {}
            for dd in dicts:
                for k, v in dd.items():
                    out.setdefault(k, []).extend(v)
            return out

        d01 = attn(0, 1, inject=merge({DRAIN_AT: [d00]}, {
            2: [lambda: proj_k_half(1, 0)],
            4: [lambda: transpose_group(1, 0)],
            7: [lambda: proj_q_half(1, 0)],
            12: [lambda: proj_k_half(1, 1)],
            14: [lambda: transpose_group(1, 1)],
        }))
        d10 = attn(1, 0, inject=merge({DRAIN_AT: [d01]},
                                      {2: [lambda: proj_q_half(1, 1)]}))
        d11 = attn(1, 1, inject={DRAIN_AT: [d10]}, drain_fast=True)
        d11()

    nc.compile()
    return nc


def _shard_inputs(inputs):
    """Build the 8 per-core input maps from the full input dict."""
    bf = mybir.dt.np(BF16)
    mw = np.asarray(inputs["modality_weights"], np.float64)
    e = np.exp(mw - mw.max())
    w = (e / e.sum()).astype(np.float64)
    w0, w1 = float(w[0]), float(w[1])

    def bn_fold(gamma, beta, mean, var, mul):
        g = np.asarray(gamma, np.float64)
        b = np.asarray(beta, np.float64)
        m = np.asarray(mean, np.float64)
        v = np.asarray(var, np.float64)
        scale = g / np.sqrt(v + EPS) * mul
        bias = (b - m * g / np.sqrt(v + EPS)) * mul
        return scale, bias

    i_s, i_b = bn_fold(inputs["image_gamma"], inputs["image_beta"],
                       inputs["image_mean"], inputs["image_var"], w0)
    l_s, l_b = bn_fold(inputs["lidar_gamma"], inputs["lidar_beta"],
                       inputs["lidar_mean"], inputs["lidar_var"], 1.0)

    # weight slices, pre-transposed for lhsT ([cin_slice, cout]), BN scale
    # folded in, bf16
    wi = (np.asarray(inputs["image_w"], np.float64).T * i_s[None, :])
    wl = (np.asarray(inputs["lidar_w"], np.float64).T * l_s[None, :])
    wi = wi.astype(np.float32).astype(bf).reshape(NCI_I, 128, CO)
    wl = wl.astype(np.float32).astype(bf).reshape(NCI_L, 128, CO)

    cb = np.zeros((128, 1024), bf)
    for ci in range(NCI_I):
        cb[:, CB_WIMG + ci * 128: CB_WIMG + (ci + 1) * 128] = wi[ci]
    for ci in range(NCI_L):
        cb[:, CB_WLID + ci * 128: CB_WLID + (ci + 1) * 128] = wl[ci]
    cb[:, CB_IDENT:CB_IDENT + 128] = np.eye(128, dtype=bf)
    cb[:, CB_INVW1:CB_INVW1 + 128] = np.full((128, 128), 1.0 / w1, bf)

    escv = 1.0 / (w0 * math.sqrt(CO))
    cfv = np.zeros((128, 8), np.float32)
    cfv[:, CF_IMG_B] = i_b.astype(np.float32)
    cfv[:, CF_LID_B] = l_b.astype(np.float32)
    cfv[:, CF_ESC] = escv
    cfv[:, CF_SCHA] = escv * SCH_A
    cfv[:, CF_SCHB] = 127.0 * 128.0 - SCH_C

    # features -> (B, C, nchunks, 2048) bf16
    img = np.asarray(inputs["image_features"], np.float32).astype(bf) \
        .reshape(B, NCI_I, 128, NCH, CHUNK)
    lid = np.asarray(inputs["lidar_features"], np.float32).astype(bf) \
        .reshape(B, NCI_L, 128, NCH, CHUNK)

    in_maps = []
    for core in range(NCORES):
        ximg = np.empty((UPC, 2, 128, NCI_I, 1024), bf)
        xlid = np.empty((UPC, 2, 128, NCI_L, 1024), bf)
        for ul in range(UPC):
            un = core * UPC + ul
            b, c = un // NCH, un % NCH
            for h in range(2):
                # [ci, 128, 1024] -> [128, ci, 1024]
                ximg[ul, h] = img[b, :, :, c, h * 1024:(h + 1) * 1024] \
                    .transpose(1, 0, 2)
                xlid[ul, h] = lid[b, :, :, c, h * 1024:(h + 1) * 1024] \
                    .transpose(1, 0, 2)
        in_maps.append({"cb": cb, "cf": cfv, "xl": xlid, "xi": ximg})
    return in_maps


def kernel(**inputs) -> np.ndarray:
    global _PROGRAM, LAST_RESULTS
    if _PROGRAM is None:
        _PROGRAM = _build_program()
    nc = _PROGRAM

    in_maps = _shard_inputs(inputs)
    trace = os.environ.get("BASS_KERNEL_TRACE", "0") == "1"
    tmpdir = os.environ.get("BASS_KERNEL_TRACE_DIR") or None
    if tmpdir:
        os.makedirs(tmpdir, exist_ok=True)
    results = run_bass_kernel_spmd(nc, in_maps, core_ids=list(range(NCORES)),
                                   trace=trace, tmpdir=tmpdir)
    LAST_RESULTS = results

    out = np.empty((B, CO, H, W), np.float32)
    outv = out.reshape(B, CO, NCH, CHUNK)
    for core in range(NCORES):
        yc = results.results[core]["y"]
        for ul in range(UPC):
            un = core * UPC + ul
            b, c = un // NCH, un % NCH
            outv[b, :, c, :] = yc[ul]
    return out


if __name__ == "__main__":
    rng = np.random.default_rng(0)
    inputs = {
        "lidar_features": rng.standard_normal((B, CL, H, W), np.float32),
        "image_features": rng.standard_normal((B, CI, H, W), np.float32),
        "lidar_w": rng.standard_normal((CO, CL), np.float32) * np.sqrt(2.0 / CO),
        "lidar_gamma": np.ones(CO, np.float32),
        "lidar_beta": np.zeros(CO, np.float32),
        "lidar_mean": rng.standard_normal(CO).astype(np.float32) * 0.1,
        "lidar_var": rng.uniform(0.5, 1.5, CO).astype(np.float32),
        "image_w": rng.standard_normal((CO, CI), np.float32) * np.sqrt(2.0 / CO),
        "image_gamma": np.ones(CO, np.float32),
        "image_beta": np.zeros(CO, np.float32),
        "image_mean": rng.standard_normal(CO).astype(np.float32) * 0.1,
        "image_var": rng.uniform(0.5, 1.5, CO).astype(np.float32),
        "modality_weights": np.ones(2, np.float32),
    }
    out = kernel(**inputs)
    print("kernel out:", out.shape, out.dtype, float(np.abs(out).mean()))
